# revision 18
# baseline (speedup 1.0000x reference)
"""Trainium2 Bass kernel for the 8-bit SNN barrel shifter.

Reference semantics (all inputs are exactly 0.0/1.0 f32):
    shift = S[:,0] + 2*S[:,1] + 4*S[:,2]
    out[:, i] = P[:, i - shift] if i >= shift else 0

Device strategy (pure data parallel over 8 cores, row-major layout):
  - host repacks P/S to uint8 bits (0/1) and shards rows across the 8 cores
  - per core the vector engine packs each row's 8 bit-bytes into one packed
    byte with a bitwise OR-tree over uint32 views (junk bits tracked >= 8),
    packs the 3 shift bits, applies one per-element logical_shift_left, and
    extracts bit pairs with single shift ops (one uint16 lane per 2 output
    bytes; each output byte holds its bit at a known position)
  - host re-interleaves the pair planes, masks the known junk bits, and
    casts back to f32
"""
import numpy as np

_N = 4194304
_CORES = 8
_NC = _N // _CORES          # rows per core
_PARTS = 128
_R = (512, 1024, 1024, 1024, 512)  # per-tile rows-per-partition schedule
# tile count follows the _R schedule
_POOL_PAIRS = 0             # how many of the 4 pair-extract ops go to GpSimd

_CACHE: dict = {}


def _build(rows_per_core: int, R, pool_pairs: int = _POOL_PAIRS, bufs: int = 3):
    import concourse.tile as tile
    from concourse import bacc, mybir

    dt = mybir.dt
    Alu = mybir.AluOpType
    P = _PARTS
    rpp = rows_per_core // P          # rows per partition
    rs = [R] * (rpp // R) if isinstance(R, int) else list(R)
    assert sum(rs) == rpp

    nc = bacc.Bacc("TRN2", target_bir_lowering=False, debug=False)
    p8 = nc.dram_tensor("p8", (rows_per_core, 8), dt.uint8, kind="ExternalInput").ap()
    s8 = nc.dram_tensor("s8", (rows_per_core, 4), dt.uint8, kind="ExternalInput").ap()
    o16 = nc.dram_tensor("o16", (rows_per_core * 4,), dt.uint16,
                         kind="ExternalOutput").ap()

    pr = p8.rearrange("(p r) c -> p r c", p=P, r=rpp)
    sr = s8.rearrange("(p r) c -> p r c", p=P, r=rpp)

    with tile.TileContext(nc) as tc:
        with tc.tile_pool(name="io", bufs=bufs) as io, tc.tile_pool(name="tmp", bufs=2) as tmp:
            r0 = 0
            for R in rs:
                pt = io.tile([P, R, 8], dt.uint8, tag="p")
                st = io.tile([P, R, 4], dt.uint8, tag="s")
                nc.sync.dma_start(pt[:], pr[:, r0:r0 + R])
                nc.scalar.dma_start(st[:], sr[:, r0:r0 + R])

                # host sends P columns permuted [0,2,4,6,1,3,5,7], so the two
                # uint32 views hold even bits / odd bits at byte positions.
                # Fold tree (junk tracked; bits 0..7 of the low half are the
                # packed byte):
                #   m = x32_odd<<1 | x32_even -> pairs at {0,1},{8,9},{16,17},{24,25}
                #   n = m>>6 | m              -> quads at {0..3}, {16..19}
                #   vi32 = n>>12 | n          -> byte at {0..7}, junk 8..13, >=16
                x32 = pt[:].bitcast(dt.uint32)          # [P, R, 2]
                m = tmp.tile([P, R], dt.uint32, tag="m")
                nc.vector.scalar_tensor_tensor(
                    m[:], x32[:, :, 1], 1, x32[:, :, 0],
                    op0=Alu.logical_shift_left, op1=Alu.bitwise_or)
                n = tmp.tile([P, R], dt.uint32, tag="n")
                nc.vector.scalar_tensor_tensor(
                    n[:], m[:], 6, m[:],
                    op0=Alu.logical_shift_right, op1=Alu.bitwise_or)
                vi32 = tmp.tile([P, R], dt.uint32, tag="vi32")
                nc.vector.scalar_tensor_tensor(
                    vi32[:], n[:], 12, n[:],
                    op0=Alu.logical_shift_right, op1=Alu.bitwise_or)
                vi = vi32[:].bitcast(dt.uint16)[:, 0::2]   # low halves, stride 2

                # pack S bits: ti = s0 + 2*s1 + 4*s2
                a = tmp.tile([P, R], dt.uint8, tag="a")
                nc.vector.scalar_tensor_tensor(
                    a[:], st[:, :, 2], 1, st[:, :, 1],
                    op0=Alu.logical_shift_left, op1=Alu.bitwise_or)
                ti = tmp.tile([P, R], dt.uint16, tag="ti")
                nc.vector.scalar_tensor_tensor(
                    ti[:], a[:], 2, st[:, :, 0],
                    op0=Alu.mult, op1=Alu.add)

                # vs = vi << ti (per-element shift, uint16)
                vs = tmp.tile([P, R], dt.uint16, tag="vs")
                nc.vector.tensor_tensor(vs[:], vi, ti[:], op=Alu.logical_shift_left)

                # extract bit pairs: lane k holds bit 2k at byte0.bit7 and
                # bit 2k+1 at byte1.bit0 (junk elsewhere, host masks)
                ot = io.tile([P, 4, R], dt.uint16, tag="o")
                for k in range(4):
                    eng = nc.gpsimd if k < pool_pairs else nc.vector
                    eng.tensor_scalar(
                        ot[:, k, :], vs[:], 7 - 2 * k, None,
                        op0=Alu.logical_shift_left)

                dst = o16[4 * P * r0: 4 * P * (r0 + R)].rearrange(
                    "(p c r) -> p c r", p=P, c=4, r=R)
                nc.scalar.dma_start(dst, ot[:])
                r0 += R
    nc.compile()
    _fix_bitwise_imms(nc, mybir)
    return nc


_BITWISE = None


def _fix_bitwise_imms(nc, mybir):
    """walrus requires integer immediates matching the src dtype on bitvec
    tensor_scalar ops; bass emits float32/int32 — rewrite them."""
    global _BITWISE
    Alu = mybir.AluOpType
    if _BITWISE is None:
        _BITWISE = {
            Alu.bitwise_and, Alu.bitwise_or, Alu.bitwise_xor, Alu.bitwise_not,
            Alu.logical_shift_left, Alu.logical_shift_right,
            Alu.arith_shift_left, Alu.arith_shift_right,
        }
    for f in nc.m.functions:
        for blk in f.blocks:
            for i in blk.instructions:
                if type(i).__name__ != "InstTensorScalarPtr":
                    continue
                ops = [getattr(i, "op0", None), getattr(i, "op1", None)]
                if not any(op in _BITWISE for op in ops if op is not None):
                    continue
                src_dt = i.ins[0].dtype
                for k in range(1, len(i.ins)):
                    iv = i.ins[k]
                    if isinstance(iv, mybir.ImmediateValue):
                        i.ins[k] = mybir.ImmediateValue(
                            dtype=src_dt, value=int(iv.value))


def _get_nc():
    key = (_NC, tuple(_R) if not isinstance(_R, int) else _R)
    if key not in _CACHE:
        _CACHE[key] = _build(*key)
    return _CACHE[key]


_PERM = [0, 2, 4, 6, 1, 3, 5, 7]


def _prep_inputs(P, S):
    Pb = np.ascontiguousarray(np.asarray(P, dtype=np.float32)[:, _PERM]).astype(np.uint8)
    s8 = np.zeros((P.shape[0], 4), np.uint8)
    s8[:, :3] = np.ascontiguousarray(S).astype(np.uint8)
    return Pb, s8


def _unshard_core(o16, rows_per_core, R):
    P = _PARTS
    rpp = rows_per_core // P
    rs = [R] * (rpp // R) if isinstance(R, int) else list(R)
    rows = np.empty((P, rpp, 8), np.uint8)
    r0 = 0
    for Rt in rs:
        chunk = o16[4 * P * r0: 4 * P * (r0 + Rt)].reshape(P, 4, Rt)
        b = chunk.view(np.uint8).reshape(P, 4, Rt, 2)
        rows[:, r0:r0 + Rt, 0::2] = ((b[..., 0] >> 7) & 1).transpose(0, 2, 1)
        rows[:, r0:r0 + Rt, 1::2] = (b[..., 1] & 1).transpose(0, 2, 1)
        r0 += Rt
    return rows.reshape(rows_per_core, 8)


def _unshard_out(o16_list):
    out = np.empty((_N, 8), np.float32)
    for c, r in enumerate(o16_list):
        out[c * _NC:(c + 1) * _NC] = _unshard_core(r.ravel(), _NC, _R)
    return out


def kernel(P: np.ndarray, S: np.ndarray) -> np.ndarray:
    from concourse.bass_utils import run_bass_kernel_spmd

    nc = _get_nc()
    Pb, s8 = _prep_inputs(P, S)
    in_maps = [
        {"p8": Pb[c * _NC:(c + 1) * _NC], "s8": s8[c * _NC:(c + 1) * _NC]}
        for c in range(_CORES)
    ]
    res = run_bass_kernel_spmd(nc, in_maps, core_ids=list(range(_CORES)))
    return _unshard_out([r["o16"] for r in res.results])


# revision 19
# speedup vs baseline: 1.0940x; 1.0940x over previous
"""Trainium2 Bass kernel for the 8-bit SNN barrel shifter.

Reference semantics (all inputs are exactly 0.0/1.0 f32):
    shift = S[:,0] + 2*S[:,1] + 4*S[:,2]
    out[:, i] = P[:, i - shift] if i >= shift else 0

Device strategy (pure data parallel over 8 cores, row-major layout):
  - host repacks P/S to uint8 bits (0/1) and shards rows across the 8 cores
  - per core the vector engine packs each row's 8 bit-bytes into one packed
    byte with a bitwise OR-tree over uint32 views (junk bits tracked >= 8),
    packs the 3 shift bits, applies one per-element logical_shift_left, and
    extracts bit pairs with single shift ops (one uint16 lane per 2 output
    bytes; each output byte holds its bit at a known position)
  - host re-interleaves the pair planes, masks the known junk bits, and
    casts back to f32
"""
import numpy as np

_N = 4194304
_CORES = 8
_NC = _N // _CORES          # rows per core
_PARTS = 128
_R = (512, 1024, 1024, 1024, 512)  # per-tile rows-per-partition schedule
# tile count follows the _R schedule
_POOL_PAIRS = 0             # how many of the 4 pair-extract ops go to GpSimd

_CACHE: dict = {}


def _build(rows_per_core: int, R, pool_pairs: int = _POOL_PAIRS, bufs: int = 3):
    import concourse.tile as tile
    from concourse import bacc, mybir

    dt = mybir.dt
    Alu = mybir.AluOpType
    P = _PARTS
    rpp = rows_per_core // P          # rows per partition
    rs = [R] * (rpp // R) if isinstance(R, int) else list(R)
    assert sum(rs) == rpp

    nc = bacc.Bacc("TRN2", target_bir_lowering=False, debug=False)
    p8 = nc.dram_tensor("p8", (rows_per_core, 8), dt.uint8, kind="ExternalInput").ap()
    s8 = nc.dram_tensor("s8", (rows_per_core, 4), dt.uint8, kind="ExternalInput").ap()
    o16 = nc.dram_tensor("o16", (rows_per_core * 4,), dt.uint16,
                         kind="ExternalOutput").ap()

    pr = p8.rearrange("(p r) c -> p r c", p=P, r=rpp)
    sr = s8.rearrange("(p r) c -> p r c", p=P, r=rpp)

    with tile.TileContext(nc) as tc:
        with tc.tile_pool(name="io", bufs=bufs) as io, tc.tile_pool(name="tmp", bufs=2) as tmp:
            r0 = 0
            for R in rs:
                pt = io.tile([P, R, 8], dt.uint8, tag="p")
                st = io.tile([P, R, 4], dt.uint8, tag="s")
                nc.sync.dma_start(pt[:], pr[:, r0:r0 + R])
                nc.sync.dma_start(st[:], sr[:, r0:r0 + R])

                # host sends P columns permuted [0,2,4,6,1,3,5,7], so the two
                # uint32 views hold even bits / odd bits at byte positions.
                # Fold tree (junk tracked; bits 0..7 of the low half are the
                # packed byte):
                #   m = x32_odd<<1 | x32_even -> pairs at {0,1},{8,9},{16,17},{24,25}
                #   n = m>>6 | m              -> quads at {0..3}, {16..19}
                #   vi32 = n>>12 | n          -> byte at {0..7}, junk 8..13, >=16
                x32 = pt[:].bitcast(dt.uint32)          # [P, R, 2]
                m = tmp.tile([P, R], dt.uint32, tag="m")
                nc.vector.scalar_tensor_tensor(
                    m[:], x32[:, :, 1], 1, x32[:, :, 0],
                    op0=Alu.logical_shift_left, op1=Alu.bitwise_or)
                n = tmp.tile([P, R], dt.uint32, tag="n")
                nc.vector.scalar_tensor_tensor(
                    n[:], m[:], 6, m[:],
                    op0=Alu.logical_shift_right, op1=Alu.bitwise_or)
                vi32 = tmp.tile([P, R], dt.uint32, tag="vi32")
                nc.vector.scalar_tensor_tensor(
                    vi32[:], n[:], 12, n[:],
                    op0=Alu.logical_shift_right, op1=Alu.bitwise_or)
                vi = vi32[:].bitcast(dt.uint16)[:, 0::2]   # low halves, stride 2

                # pack S bits: ti = s0 + 2*s1 + 4*s2
                a = tmp.tile([P, R], dt.uint8, tag="a")
                nc.vector.scalar_tensor_tensor(
                    a[:], st[:, :, 2], 1, st[:, :, 1],
                    op0=Alu.logical_shift_left, op1=Alu.bitwise_or)
                ti = tmp.tile([P, R], dt.uint16, tag="ti")
                nc.vector.scalar_tensor_tensor(
                    ti[:], a[:], 2, st[:, :, 0],
                    op0=Alu.mult, op1=Alu.add)

                # vs = vi << ti (per-element shift, uint16)
                vs = tmp.tile([P, R], dt.uint16, tag="vs")
                nc.vector.tensor_tensor(vs[:], vi, ti[:], op=Alu.logical_shift_left)

                # extract bit pairs: lane k holds bit 2k at byte0.bit7 and
                # bit 2k+1 at byte1.bit0 (junk elsewhere, host masks)
                ot = io.tile([P, 4, R], dt.uint16, tag="o")
                for k in range(4):
                    eng = nc.gpsimd if k < pool_pairs else nc.vector
                    eng.tensor_scalar(
                        ot[:, k, :], vs[:], 7 - 2 * k, None,
                        op0=Alu.logical_shift_left)

                dst = o16[4 * P * r0: 4 * P * (r0 + R)].rearrange(
                    "(p c r) -> p c r", p=P, c=4, r=R)
                nc.scalar.dma_start(dst, ot[:])
                r0 += R
    nc.compile()
    _fix_bitwise_imms(nc, mybir)
    return nc


_BITWISE = None


def _fix_bitwise_imms(nc, mybir):
    """walrus requires integer immediates matching the src dtype on bitvec
    tensor_scalar ops; bass emits float32/int32 — rewrite them."""
    global _BITWISE
    Alu = mybir.AluOpType
    if _BITWISE is None:
        _BITWISE = {
            Alu.bitwise_and, Alu.bitwise_or, Alu.bitwise_xor, Alu.bitwise_not,
            Alu.logical_shift_left, Alu.logical_shift_right,
            Alu.arith_shift_left, Alu.arith_shift_right,
        }
    for f in nc.m.functions:
        for blk in f.blocks:
            for i in blk.instructions:
                if type(i).__name__ != "InstTensorScalarPtr":
                    continue
                ops = [getattr(i, "op0", None), getattr(i, "op1", None)]
                if not any(op in _BITWISE for op in ops if op is not None):
                    continue
                src_dt = i.ins[0].dtype
                for k in range(1, len(i.ins)):
                    iv = i.ins[k]
                    if isinstance(iv, mybir.ImmediateValue):
                        i.ins[k] = mybir.ImmediateValue(
                            dtype=src_dt, value=int(iv.value))


def _get_nc():
    key = (_NC, tuple(_R) if not isinstance(_R, int) else _R)
    if key not in _CACHE:
        _CACHE[key] = _build(*key)
    return _CACHE[key]


_PERM = [0, 2, 4, 6, 1, 3, 5, 7]


def _prep_inputs(P, S):
    Pb = np.ascontiguousarray(np.asarray(P, dtype=np.float32)[:, _PERM]).astype(np.uint8)
    s8 = np.zeros((P.shape[0], 4), np.uint8)
    s8[:, :3] = np.ascontiguousarray(S).astype(np.uint8)
    return Pb, s8


def _unshard_core(o16, rows_per_core, R):
    P = _PARTS
    rpp = rows_per_core // P
    rs = [R] * (rpp // R) if isinstance(R, int) else list(R)
    rows = np.empty((P, rpp, 8), np.uint8)
    r0 = 0
    for Rt in rs:
        chunk = o16[4 * P * r0: 4 * P * (r0 + Rt)].reshape(P, 4, Rt)
        b = chunk.view(np.uint8).reshape(P, 4, Rt, 2)
        rows[:, r0:r0 + Rt, 0::2] = ((b[..., 0] >> 7) & 1).transpose(0, 2, 1)
        rows[:, r0:r0 + Rt, 1::2] = (b[..., 1] & 1).transpose(0, 2, 1)
        r0 += Rt
    return rows.reshape(rows_per_core, 8)


def _unshard_out(o16_list):
    out = np.empty((_N, 8), np.float32)
    for c, r in enumerate(o16_list):
        out[c * _NC:(c + 1) * _NC] = _unshard_core(r.ravel(), _NC, _R)
    return out


def kernel(P: np.ndarray, S: np.ndarray) -> np.ndarray:
    from concourse.bass_utils import run_bass_kernel_spmd

    nc = _get_nc()
    Pb, s8 = _prep_inputs(P, S)
    in_maps = [
        {"p8": Pb[c * _NC:(c + 1) * _NC], "s8": s8[c * _NC:(c + 1) * _NC]}
        for c in range(_CORES)
    ]
    res = run_bass_kernel_spmd(nc, in_maps, core_ids=list(range(_CORES)))
    return _unshard_out([r["o16"] for r in res.results])


# revision 21
# speedup vs baseline: 1.1301x; 1.0330x over previous
"""Trainium2 Bass kernel for the 8-bit SNN barrel shifter.

Reference semantics (all inputs are exactly 0.0/1.0 f32):
    shift = S[:,0] + 2*S[:,1] + 4*S[:,2]
    out[:, i] = P[:, i - shift] if i >= shift else 0

Device strategy (pure data parallel over 8 cores, row-major layout):
  - host repacks P/S to uint8 bits (0/1) and shards rows across the 8 cores
  - per core the vector engine packs each row's 8 bit-bytes into one packed
    byte with a bitwise OR-tree over uint32 views (junk bits tracked >= 8),
    packs the 3 shift bits, applies one per-element logical_shift_left, and
    extracts bit pairs with single shift ops (one uint16 lane per 2 output
    bytes; each output byte holds its bit at a known position)
  - host re-interleaves the pair planes, masks the known junk bits, and
    casts back to f32
"""
import numpy as np

_N = 4194304
_CORES = 8
_NC = _N // _CORES          # rows per core
_PARTS = 128
_R = (512, 1024, 1024, 1024, 512)  # per-tile rows-per-partition schedule
# tile count follows the _R schedule
_POOL_PAIRS = 0             # how many of the 4 pair-extract ops go to GpSimd

_CACHE: dict = {}


def _build(rows_per_core: int, R, pool_pairs: int = _POOL_PAIRS, bufs: int = 3):
    import concourse.tile as tile
    from concourse import bacc, mybir

    dt = mybir.dt
    Alu = mybir.AluOpType
    P = _PARTS
    rpp = rows_per_core // P          # rows per partition
    rs = [R] * (rpp // R) if isinstance(R, int) else list(R)
    assert sum(rs) == rpp

    nc = bacc.Bacc("TRN2", target_bir_lowering=False, debug=False)
    p8 = nc.dram_tensor("p8", (rows_per_core, 8), dt.uint8, kind="ExternalInput").ap()
    s8 = nc.dram_tensor("s8", (rows_per_core, 4), dt.uint8, kind="ExternalInput").ap()
    o16 = nc.dram_tensor("o16", (rows_per_core * 4,), dt.uint16,
                         kind="ExternalOutput").ap()

    pr = p8.rearrange("(p r) c -> p r c", p=P, r=rpp)
    sr = s8.rearrange("(p r) c -> p r c", p=P, r=rpp)

    with tile.TileContext(nc) as tc:
        with tc.tile_pool(name="io", bufs=bufs) as io, tc.tile_pool(name="tmp", bufs=2) as tmp:
            r0 = 0
            for R in rs:
                pt = io.tile([P, R, 8], dt.uint8, tag="p")
                st = io.tile([P, R, 4], dt.uint8, tag="s")
                nc.sync.dma_start(pt[:], pr[:, r0:r0 + R])
                nc.sync.dma_start(st[:], sr[:, r0:r0 + R])

                # host sends P columns permuted [0,2,4,6,1,3,5,7], so the two
                # uint32 views hold even bits / odd bits at byte positions.
                # Fold tree (junk tracked; bits 0..7 of the low half are the
                # packed byte):
                #   m = x32_odd<<1 | x32_even -> pairs at {0,1},{8,9},{16,17},{24,25}
                #   n = m>>6 | m              -> quads at {0..3}, {16..19}
                #   vi32 = n>>12 | n          -> byte at {0..7}, junk 8..13, >=16
                x32 = pt[:].bitcast(dt.uint32)          # [P, R, 2]
                m = tmp.tile([P, R], dt.uint32, tag="m")
                nc.vector.scalar_tensor_tensor(
                    m[:], x32[:, :, 1], 1, x32[:, :, 0],
                    op0=Alu.logical_shift_left, op1=Alu.bitwise_or)
                n = tmp.tile([P, R], dt.uint32, tag="n")
                nc.vector.scalar_tensor_tensor(
                    n[:], m[:], 6, m[:],
                    op0=Alu.logical_shift_right, op1=Alu.bitwise_or)
                # final fold on uint16 views of n: even halves hold the low
                # quad, odd halves the high quad -> dense uint16 vi
                n16 = n[:].bitcast(dt.uint16)           # [P, 2R]
                vi = tmp.tile([P, R], dt.uint16, tag="vi")
                nc.vector.scalar_tensor_tensor(
                    vi[:], n16[:, 1::2], 4, n16[:, 0::2],
                    op0=Alu.logical_shift_left, op1=Alu.bitwise_or)

                # pack S bits: ti = s0 + 2*s1 + 4*s2
                a = tmp.tile([P, R], dt.uint8, tag="a")
                nc.vector.scalar_tensor_tensor(
                    a[:], st[:, :, 2], 1, st[:, :, 1],
                    op0=Alu.logical_shift_left, op1=Alu.bitwise_or)
                ti = tmp.tile([P, R], dt.uint16, tag="ti")
                nc.vector.scalar_tensor_tensor(
                    ti[:], a[:], 2, st[:, :, 0],
                    op0=Alu.mult, op1=Alu.add)

                # vs = vi << ti (per-element shift, uint16)
                vs = tmp.tile([P, R], dt.uint16, tag="vs")
                nc.vector.tensor_tensor(vs[:], vi[:], ti[:], op=Alu.logical_shift_left)

                # extract bit pairs: lane k holds bit 2k at byte0.bit7 and
                # bit 2k+1 at byte1.bit0 (junk elsewhere, host masks)
                ot = io.tile([P, 4, R], dt.uint16, tag="o")
                for k in range(4):
                    eng = nc.gpsimd if k < pool_pairs else nc.vector
                    eng.tensor_scalar(
                        ot[:, k, :], vs[:], 7 - 2 * k, None,
                        op0=Alu.logical_shift_left)

                dst = o16[4 * P * r0: 4 * P * (r0 + R)].rearrange(
                    "(p c r) -> p c r", p=P, c=4, r=R)
                nc.scalar.dma_start(dst, ot[:])
                r0 += R
    nc.compile()
    _fix_bitwise_imms(nc, mybir)
    return nc


_BITWISE = None


def _fix_bitwise_imms(nc, mybir):
    """walrus requires integer immediates matching the src dtype on bitvec
    tensor_scalar ops; bass emits float32/int32 — rewrite them."""
    global _BITWISE
    Alu = mybir.AluOpType
    if _BITWISE is None:
        _BITWISE = {
            Alu.bitwise_and, Alu.bitwise_or, Alu.bitwise_xor, Alu.bitwise_not,
            Alu.logical_shift_left, Alu.logical_shift_right,
            Alu.arith_shift_left, Alu.arith_shift_right,
        }
    for f in nc.m.functions:
        for blk in f.blocks:
            for i in blk.instructions:
                if type(i).__name__ != "InstTensorScalarPtr":
                    continue
                ops = [getattr(i, "op0", None), getattr(i, "op1", None)]
                if not any(op in _BITWISE for op in ops if op is not None):
                    continue
                src_dt = i.ins[0].dtype
                for k in range(1, len(i.ins)):
                    iv = i.ins[k]
                    if isinstance(iv, mybir.ImmediateValue):
                        i.ins[k] = mybir.ImmediateValue(
                            dtype=src_dt, value=int(iv.value))


def _get_nc():
    key = (_NC, tuple(_R) if not isinstance(_R, int) else _R)
    if key not in _CACHE:
        _CACHE[key] = _build(*key)
    return _CACHE[key]


_PERM = [0, 2, 4, 6, 1, 3, 5, 7]


def _prep_inputs(P, S):
    Pb = np.ascontiguousarray(np.asarray(P, dtype=np.float32)[:, _PERM]).astype(np.uint8)
    s8 = np.zeros((P.shape[0], 4), np.uint8)
    s8[:, :3] = np.ascontiguousarray(S).astype(np.uint8)
    return Pb, s8


def _unshard_core(o16, rows_per_core, R):
    P = _PARTS
    rpp = rows_per_core // P
    rs = [R] * (rpp // R) if isinstance(R, int) else list(R)
    rows = np.empty((P, rpp, 8), np.uint8)
    r0 = 0
    for Rt in rs:
        chunk = o16[4 * P * r0: 4 * P * (r0 + Rt)].reshape(P, 4, Rt)
        b = chunk.view(np.uint8).reshape(P, 4, Rt, 2)
        rows[:, r0:r0 + Rt, 0::2] = ((b[..., 0] >> 7) & 1).transpose(0, 2, 1)
        rows[:, r0:r0 + Rt, 1::2] = (b[..., 1] & 1).transpose(0, 2, 1)
        r0 += Rt
    return rows.reshape(rows_per_core, 8)


def _unshard_out(o16_list):
    out = np.empty((_N, 8), np.float32)
    for c, r in enumerate(o16_list):
        out[c * _NC:(c + 1) * _NC] = _unshard_core(r.ravel(), _NC, _R)
    return out


def kernel(P: np.ndarray, S: np.ndarray) -> np.ndarray:
    from concourse.bass_utils import run_bass_kernel_spmd

    nc = _get_nc()
    Pb, s8 = _prep_inputs(P, S)
    in_maps = [
        {"p8": Pb[c * _NC:(c + 1) * _NC], "s8": s8[c * _NC:(c + 1) * _NC]}
        for c in range(_CORES)
    ]
    res = run_bass_kernel_spmd(nc, in_maps, core_ids=list(range(_CORES)))
    return _unshard_out([r["o16"] for r in res.results])
